# revision 12
# baseline (speedup 1.0000x reference)
"""Trainium2 Bass kernel for nn_EquivariantResidualLayer (GNN message passing).

Strategy (8-core SPMD, no collectives):
  - Edges are sharded by TARGET node range: core c owns targets
    [c*6250, (c+1)*6250). Host sorts each core's edges by target window
    (128 targets per window) so the segment-sum becomes a per-window
    one-hot matmul accumulating in PSUM. No cross-core reduction needed.
  - Source-node features (post-LN scalar state, vector state) are gathered
    with dma_gather from per-core HBM copies. The node table is split in two
    halves (< 32768 rows each) because gather indices are int16.
  - LayerNorm affine (g, b) is folded into the MLP weights on the host;
    the device computes the plain standardization once per core.
  - The target-side contribution of the edge MLP is fused: per window,
    G_w = s_norm[window] @ W1_tgt is precomputed (128x320), and each edge
    tile's target contribution is onehot^T-matmul'd from G_w. This avoids
    both the target gather and its transpose.
  - Node MLP runs per window of 128 nodes right after its window's edge
    aggregation completes (data-parallel over nodes).

kernel(**inputs) takes FULL inputs, returns (scalar_state, vector_state).
"""

import math
import os
import numpy as np


# ---------------------------------------------------------------------------
# configuration
# ---------------------------------------------------------------------------

class Cfg:
    def __init__(self, N=50000, E=800000, H=128, V=32, ED=64, ncores=8,
                 half=25088, W=128, TB=4):
        self.N, self.E, self.H, self.V, self.ED = N, E, H, V, ED
        self.ncores = ncores
        self.W = W                      # targets per window
        self.TB = TB                    # 128-blocks per processing tile
        self.npc = N // ncores          # nodes per core (6250)
        assert self.npc * ncores == N
        self.nwin = math.ceil(self.npc / W)          # windows per core (49)
        self.nloc = self.nwin * W                    # padded local nodes (6272)
        # full padded node-table length (gather + LN source):
        need = (ncores - 1) * self.npc + self.nloc
        self.npadt = ((max(N, need) + 127) // 128) * 128
        self.half = half                # node-table split point (rows < 32768 each)
        assert self.half < 32768 and self.npadt - self.half < 32768
        self.EH = 2 * H + 2 * V         # edge MLP hidden (320)
        self.EO = 2 * H + 2 * V         # edge MLP out (320)
        self.NH = 2 * H + V             # node MLP hidden (288)
        self.NO = H + V                 # node MLP out (160)
        self.F3 = 3 * V                 # flattened vector feature (96)
        self.VP = H                     # padded vector row (gather wants 512B rows)


# ---------------------------------------------------------------------------
# host-side prep: shard edges, build per-core index/payload arrays
# ---------------------------------------------------------------------------

def _roundup(x, m):
    return ((x + m - 1) // m) * m


def host_prep(inputs, cfg: Cfg):
    H, V, ED, W = cfg.H, cfg.V, cfg.ED, cfg.W
    NC, npc, nwin = cfg.ncores, cfg.npc, cfg.nwin

    ei = np.asarray(inputs["edge_index"])
    tgt = ei[0].astype(np.int64)
    src = ei[1].astype(np.int64)
    edge_attr = np.asarray(inputs["edge_attr"], dtype=np.float32)
    edge_vector = np.asarray(inputs["edge_vector"], dtype=np.float32)
    scalar_state = np.asarray(inputs["scalar_state"], dtype=np.float32)
    vector_state = np.asarray(inputs["vector_state"], dtype=np.float32)

    # ---- fold LN affine into MLP weights -------------------------------
    e_w1 = np.asarray(inputs["e_w1"], np.float32)
    e_b1 = np.asarray(inputs["e_b1"], np.float32)
    sn_g = np.asarray(inputs["sn_g"], np.float32)
    sn_b = np.asarray(inputs["sn_b"], np.float32)
    w1 = e_w1.copy()
    w1[0:H] = sn_g[:, None] * w1[0:H]
    w1[H:2 * H] = sn_g[:, None] * w1[H:2 * H]
    b1 = e_b1 + sn_b @ e_w1[0:H] + sn_b @ e_w1[H:2 * H]

    n_w1 = np.asarray(inputs["n_w1"], np.float32)
    n_b1 = np.asarray(inputs["n_b1"], np.float32)
    pn_g = np.asarray(inputs["pn_g"], np.float32)
    pn_b = np.asarray(inputs["pn_b"], np.float32)
    nw1 = n_w1.copy()
    nw1[0:H] = pn_g[:, None] * nw1[0:H]
    nb1 = n_b1 + pn_b @ n_w1[0:H]

    e_w2 = np.asarray(inputs["e_w2"], np.float32)
    e_b2 = np.asarray(inputs["e_b2"], np.float32)
    n_w2 = np.asarray(inputs["n_w2"], np.float32)
    n_b2 = np.asarray(inputs["n_b2"], np.float32)

    # e_b2 is applied to `upd` before splitting. Fold it into the edge MLP
    # via the message pipeline: upd = h@W2 + b2. We add b2 on-device via the
    # silu bias trick only for b1; for b2 we append a homogeneous row: the
    # device adds b2 with a ones-row matmul per subtile. Simpler: bake b2
    # into W2 with an extra hidden row fed by a constant-1 h row. We instead
    # handle b2 on device with a rank-1 matmul (ones ^T b2) per upd tile.

    # ---- padded full node tables ---------------------------------------
    npadt = cfg.npadt
    sc_pad = np.zeros((npadt, H), np.float32)
    sc_pad[:cfg.N] = scalar_state
    vec_pad = np.zeros((npadt, cfg.VP), np.float32)
    vec_pad[:cfg.N, :cfg.F3] = vector_state.reshape(cfg.N, cfg.F3)

    # ---- per-core edge partition & window grouping ---------------------
    core_of = tgt // npc
    per_core = []
    counts_L = np.zeros((NC, nwin), np.int64)
    counts_H = np.zeros((NC, nwin), np.int64)
    for c in range(NC):
        sel = np.nonzero(core_of == c)[0]
        loc = tgt[sel] - c * npc
        win = loc // W
        rel = loc % W
        grp = (src[sel] >= cfg.half).astype(np.int64)
        order = np.lexsort((sel, grp, win))
        sel, loc, win, rel, grp = (a[order] for a in (sel, loc, win, rel, grp))
        for w in range(nwin):
            m = win == w
            counts_L[c, w] = np.count_nonzero(m & (grp == 0))
            counts_H[c, w] = np.count_nonzero(m & (grp == 1))
        per_core.append((sel, win, rel, grp))

    CL = np.maximum(_roundup(counts_L.max(axis=0), 128), 128).astype(np.int64)
    CH = np.maximum(_roundup(counts_H.max(axis=0), 128), 128).astype(np.int64)
    caps = CL + CH
    nblk = caps // 128
    slot_off = np.concatenate([[0], np.cumsum(caps)])
    blk_off = np.concatenate([[0], np.cumsum(nblk)])
    icolsL = CL // 16
    icolsH = CH // 16
    icols = icolsL + icolsH
    icol_off = np.concatenate([[0], np.cumsum(icols)])
    S_tot = int(slot_off[-1])
    NBLK_tot = int(blk_off[-1])
    ICOL_tot = int(icol_off[-1])

    meta = dict(CL=CL, CH=CH, caps=caps, nblk=nblk, slot_off=slot_off,
                blk_off=blk_off, icolsL=icolsL, icolsH=icolsH,
                icol_off=icol_off, S_tot=S_tot, NBLK_tot=NBLK_tot,
                ICOL_tot=ICOL_tot)

    # ---- per-core arrays ------------------------------------------------
    in_maps = []
    for c in range(NC):
        sel, win, rel, grp = per_core[c]
        # rank within (window, group)
        slot = np.empty(len(sel), np.int64)
        for w in range(nwin):
            mL = (win == w) & (grp == 0)
            mH = (win == w) & (grp == 1)
            base = slot_off[w]
            slot[mL] = base + np.arange(np.count_nonzero(mL))
            slot[mH] = base + CL[w] + np.arange(np.count_nonzero(mH))

        attr_t = np.zeros((ED, S_tot), np.float32)
        attr_t[:, slot] = edge_attr[sel].T
        evec_h = np.zeros((128, NBLK_tot, 3), np.float32)
        evec_h[slot % 128, slot // 128, :] = edge_vector[sel]
        wrow = np.full((1, S_tot), -1.0, np.float32)
        wrow[0, slot] = rel.astype(np.float32)
        wcol = np.full((128, NBLK_tot), -1.0, np.float32)
        wcol[slot % 128, slot // 128] = rel.astype(np.float32)

        idx16 = np.zeros((16, ICOL_tot), np.int16)
        srcs = src[sel]
        for w in range(nwin):
            mL = (win == w) & (grp == 0)
            mH = (win == w) & (grp == 1)
            vL = np.zeros(CL[w], np.int64)
            vL[:np.count_nonzero(mL)] = srcs[mL]
            vH = np.zeros(CH[w], np.int64)
            vH[:np.count_nonzero(mH)] = srcs[mH] - cfg.half
            o = icol_off[w]
            idx16[:, o:o + icolsL[w]] = vL.reshape(icolsL[w], 16).T
            idx16[:, o + icolsL[w]:o + icols[w]] = vH.reshape(icolsH[w], 16).T
        idx128 = np.tile(idx16, (8, 1))

        lo = c * npc
        sc_local = np.zeros((cfg.nloc, H), np.float32)
        n_here = min(cfg.nloc, cfg.N - lo)
        sc_local[:n_here] = scalar_state[lo:lo + n_here]
        vec_local = np.zeros((cfg.nloc, cfg.F3), np.float32)
        vec_local[:n_here] = vector_state.reshape(cfg.N, cfg.F3)[lo:lo + n_here]

        # packed weights (see build_program for layouts)
        w2_pack = np.zeros((128, 3 * cfg.EO), np.float32)
        w2_pack[:, 0:320] = e_w2[0:128]
        w2_pack[:, 320:640] = e_w2[128:256]
        w2_pack[0:64, 640:960] = e_w2[256:320]
        nw1_pack = np.zeros((128, 2 * cfg.NH), np.float32)
        nw1_pack[:, 0:cfg.NH] = nw1[0:128]
        nw1_pack[0:cfg.V, cfg.NH:] = nw1[128:160]
        nw2_pack = np.zeros((128, 3 * cfg.NO), np.float32)
        nw2_pack[:, 0:160] = n_w2[0:128]
        nw2_pack[:, 160:320] = n_w2[128:256]
        nw2_pack[0:cfg.NH - 256, 320:480] = n_w2[256:cfg.NH]

        in_maps.append({
            "sc_pad": sc_pad, "vec_pad": vec_pad,
            "sc_local": sc_local, "vec_local": vec_local,
            "attr_t": attr_t, "evec_h": evec_h,
            "wrow": wrow, "wcol": wcol, "idx16": idx128,
            "w1a": np.ascontiguousarray(w1[0:H]),
            "w1b": np.ascontiguousarray(w1[H:2 * H]),
            "w1c": np.ascontiguousarray(w1[2 * H:]),
            "b1": b1.reshape(-1, 1).copy(),
            "w2p": w2_pack, "b2": e_b2.reshape(1, -1).copy(),
            "nw1p": nw1_pack, "nb1": nb1.reshape(-1, 1).copy(),
            "nw2p": nw2_pack, "nb2": n_b2.reshape(1, -1).copy(),
        })

    return in_maps, meta


# ---------------------------------------------------------------------------
# device program
# ---------------------------------------------------------------------------

def build_program(cfg: Cfg, meta):
    import concourse.bass as bass
    import concourse.tile as tile
    from concourse import mybir
    from concourse.masks import make_identity

    f32 = mybir.dt.float32
    i32 = mybir.dt.int32
    i16 = mybir.dt.int16
    AF = mybir.ActivationFunctionType
    OP = mybir.AluOpType
    AX = mybir.AxisListType

    H, V, ED = cfg.H, cfg.V, cfg.ED
    EH, EO, NH, NO, F3, VP = cfg.EH, cfg.EO, cfg.NH, cfg.NO, cfg.F3, cfg.VP
    nwin, nloc, npadt = cfg.nwin, cfg.nloc, cfg.npadt
    TB = cfg.TB
    CL, CH, caps, nblk = meta["CL"], meta["CH"], meta["caps"], meta["nblk"]
    slot_off, blk_off = meta["slot_off"], meta["blk_off"]
    icolsL, icolsH, icol_off = meta["icolsL"], meta["icolsH"], meta["icol_off"]
    S_tot, NBLK_tot, ICOL_tot = meta["S_tot"], meta["NBLK_tot"], meta["ICOL_tot"]
    max_nblk = int(nblk.max())
    max_cap = int(caps.max())
    max_icols = int((icolsL + icolsH).max())

    # hidden chunking of the 320-wide edge MLP dims
    CHK = [(0, 128), (128, 128), (256, 64)]
    # node MLP hidden chunks (288)
    NCHK = [(0, 128), (128, 128), (256, 32)]

    from concourse.bacc import Bacc
    nc = Bacc()

    # ---- DRAM I/O ------------------------------------------------------
    din = lambda n, s, d=f32: nc.dram_tensor(n, s, d, kind="ExternalInput")
    sc_pad = din("sc_pad", [npadt, H])
    vec_pad = din("vec_pad", [npadt, VP])
    sc_local = din("sc_local", [nloc, H])
    vec_local = din("vec_local", [nloc, F3])
    attr_t = din("attr_t", [ED, S_tot])
    evec_h = din("evec_h", [128, NBLK_tot, 3])
    wrow = din("wrow", [1, S_tot])
    wcol = din("wcol", [128, NBLK_tot])
    idx16 = din("idx16", [128, ICOL_tot], i16)
    w1a_d = din("w1a", [H, EH])
    w1b_d = din("w1b", [H, EH])
    w1c_d = din("w1c", [ED, EH])
    b1_d = din("b1", [EH, 1])
    w2p_d = din("w2p", [128, 3 * EO])
    b2_d = din("b2", [1, EO])
    nw1p_d = din("nw1p", [128, 2 * NH])
    nb1_d = din("nb1", [NH, 1])
    nw2p_d = din("nw2p", [128, 3 * NO])
    nb2_d = din("nb2", [1, NO])

    s_out = nc.dram_tensor("s_out", [nloc, H], f32, kind="ExternalOutput")
    v_out = nc.dram_tensor("v_out", [nloc, F3], f32, kind="ExternalOutput")

    s_norm = nc.dram_tensor("s_norm_scratch", [npadt, H], f32, kind="Internal")

    ntile_ln = npadt // 128

    with tile.TileContext(nc) as tc:
        with (
            tc.tile_pool(name="const", bufs=1) as cpool,
            tc.tile_pool(name="win", bufs=2) as wpool,
            tc.tile_pool(name="work", bufs=2) as kpool,
            tc.tile_pool(name="oh", bufs=3) as ohpool,
            tc.tile_pool(name="ln", bufs=2) as lnpool,
            tc.tile_pool(name="ps_big", bufs=3, space="PSUM") as pbig,
            tc.tile_pool(name="ps_upd", bufs=2, space="PSUM") as pupd,
            tc.tile_pool(name="ps_acc", bufs=1, space="PSUM") as pacc,
        ):
            # ---- constants -------------------------------------------
            ident = cpool.tile([128, 128], f32, tag="ident")
            make_identity(nc, ident[:])
            iota_i = cpool.tile([128, 128], i32, tag="iota_i")
            nc.gpsimd.iota(iota_i[:], pattern=[[1, 128]], base=0,
                           channel_multiplier=0)
            iota_row = cpool.tile([128, 128], f32, tag="iota_row")
            nc.vector.tensor_copy(iota_row[:], iota_i[:])
            iota_ci = cpool.tile([128, 1], i32, tag="iota_ci")
            nc.gpsimd.iota(iota_ci[:], pattern=[[0, 1]], base=0,
                           channel_multiplier=1)
            iota_col = cpool.tile([128, 1], f32, tag="iota_col")
            nc.vector.tensor_copy(iota_col[:], iota_ci[:])
            ones_col = cpool.tile([128, 1], f32, tag="ones_col")
            nc.vector.memset(ones_col[:], 1.0)
            ones_row = cpool.tile([1, 128], f32, tag="ones_row")
            nc.vector.memset(ones_row[:], 1.0)
            eps_col = cpool.tile([128, 1], f32, tag="eps_col")
            nc.vector.memset(eps_col[:], 1e-5)

            # ---- weights ---------------------------------------------
            w1a = cpool.tile([H, EH], f32, tag="w1a")
            nc.sync.dma_start(w1a[:], w1a_d[:])
            w1b = cpool.tile([H, EH], f32, tag="w1b")
            nc.sync.dma_start(w1b[:], w1b_d[:])
            w1c = cpool.tile([ED, EH], f32, tag="w1c")
            nc.sync.dma_start(w1c[:], w1c_d[:])
            b1c = cpool.tile([128, 3], f32, tag="b1c")
            for j, (mo, mw) in enumerate(CHK):
                nc.sync.dma_start(b1c[0:mw, j:j + 1], b1_d[mo:mo + mw, :])
            w2p = cpool.tile([128, 3 * EO], f32, tag="w2p")
            nc.sync.dma_start(w2p[:], w2p_d[:])
            b2r = cpool.tile([1, EO], f32, tag="b2r")
            nc.sync.dma_start(b2r[:], b2_d[:])
            nw1p = cpool.tile([128, 2 * NH], f32, tag="nw1p")
            nc.sync.dma_start(nw1p[:], nw1p_d[:])
            nb1c = cpool.tile([128, 3], f32, tag="nb1c")
            for j, (mo, mw) in enumerate(NCHK):
                nc.sync.dma_start(nb1c[0:mw, j:j + 1], nb1_d[mo:mo + mw, :])
            nw2p = cpool.tile([128, 3 * NO], f32, tag="nw2p")
            nc.sync.dma_start(nw2p[:], nw2p_d[:])
            nb2r = cpool.tile([1, NO], f32, tag="nb2r")
            nc.sync.dma_start(nb2r[:], nb2_d[:])

            # ---- helper: layer-norm stats+apply for one [128, nt, 128]
            def ln_block(x, nt, out_tiles, act_share=True):
                # x: SBUF [128, nt, 128]; out_tiles: list of (dest_ap per t)
                s = lnpool.tile([128, nt, 1], f32, tag="ln_s")
                nc.vector.tensor_reduce(s[:], x[:, 0:nt, :], axis=AX.X, op=OP.add)
                sq = lnpool.tile([128, nt, 128], f32, tag="ln_sq")
                nc.vector.tensor_tensor(sq[:, 0:nt, :], x[:, 0:nt, :],
                                        x[:, 0:nt, :], op=OP.mult)
                ss = lnpool.tile([128, nt, 1], f32, tag="ln_ss")
                nc.vector.tensor_reduce(ss[:], sq[:, 0:nt, :], axis=AX.X,
                                        op=OP.add)
                mean = lnpool.tile([128, nt], f32, tag="ln_mean")
                nc.vector.tensor_scalar_mul(mean[:], s[:, 0:nt, 0], 1.0 / 128.0)
                var = lnpool.tile([128, nt], f32, tag="ln_var")
                # var = ss/128 - mean^2
                nc.vector.tensor_scalar_mul(var[:], ss[:, 0:nt, 0], 1.0 / 128.0)
                m2 = lnpool.tile([128, nt], f32, tag="ln_m2")
                nc.vector.tensor_tensor(m2[:], mean[:], mean[:], op=OP.mult)
                nc.vector.tensor_tensor(var[:], var[:], m2[:], op=OP.subtract)
                std = lnpool.tile([128, nt], f32, tag="ln_std")
                nc.scalar.activation(std[:], var[:], AF.Sqrt, bias=eps_col[:])
                inv = lnpool.tile([128, nt], f32, tag="ln_inv")
                nc.vector.reciprocal(inv[:], std[:])
                nmi = lnpool.tile([128, nt], f32, tag="ln_nmi")
                nc.vector.scalar_tensor_tensor(nmi[:], mean[:], -1.0, inv[:],
                                               op0=OP.mult, op1=OP.mult)
                for t in range(nt):
                    dest = out_tiles[t]
                    if act_share and t % 2 == 0:
                        nc.scalar.activation(dest, x[:, t, :], AF.Identity,
                                             bias=nmi[:, t:t + 1],
                                             scale=inv[:, t:t + 1])
                    else:
                        nc.vector.scalar_tensor_tensor(
                            dest, x[:, t, :], inv[:, t:t + 1],
                            nmi[:, t:t + 1].to_broadcast([128, 128]),
                            op0=OP.mult, op1=OP.add)
                return inv

            # ---- phase 1: full-table LN → s_norm ---------------------
            NT = 8
            ln_stores = []
            for g in range(0, ntile_ln, NT):
                nt = min(NT, ntile_ln - g)
                x = lnpool.tile([128, NT, 128], f32, tag="ln_x")
                nc.sync.dma_start(
                    x[:, 0:nt, :],
                    sc_pad.rearrange("(n p) d -> p n d", p=128)[:, g:g + nt, :])
                y = lnpool.tile([128, NT, 128], f32, tag="ln_y")
                ln_block(x, nt, [y[:, t, :] for t in range(nt)])
                ln_stores.append(nc.sync.dma_start(
                    s_norm.rearrange("(n p) d -> p n d", p=128)[:, g:g + nt, :],
                    y[:, 0:nt, :]))

            # ---- phase 2: local transposed s_norm (feature-major) ----
            localT = cpool.tile([128, nloc], f32, tag="localT")
            for w in range(nwin):
                x = lnpool.tile([128, 1, 128], f32, tag="loc_x")
                nc.sync.dma_start(x[:, 0, :],
                                  sc_local[w * 128:(w + 1) * 128, :])
                yloc = lnpool.tile([128, 128], f32, tag="loc_y")
                ln_block(x, 1, [yloc[:]])
                tp = pbig.tile([128, 128], f32, tag="big")
                nc.tensor.transpose(tp[:], yloc[:], ident[:])
                nc.scalar.copy(localT[:, w * 128:(w + 1) * 128], tp[:])

            # ---- phase 3: edge windows -------------------------------
            snA = s_norm[0:cfg.half, :]
            snB = s_norm[cfg.half:npadt, :]
            vpA = vec_pad[0:cfg.half, :]
            vpB = vec_pad[cfg.half:npadt, :]

            for w in range(nwin):
                cw, nb = int(caps[w]), int(nblk[w])
                cl, chh = int(CL[w]), int(CH[w])
                clb = cl // 128
                so, bo, io = int(slot_off[w]), int(blk_off[w]), int(icol_off[w])
                icl, ich = int(icolsL[w]), int(icolsH[w])

                # G_w = s_norm_win @ W1a  (targets x 320)
                gwp = pbig.tile([128, EH], f32, tag="big")
                nc.tensor.matmul(gwp[:], localT[:, w * 128:(w + 1) * 128],
                                 w1a[:], start=True, stop=True)
                gw = wpool.tile([128, EH], f32, tag="gw")
                nc.scalar.copy(gw[:], gwp[:])

                # window data loads
                idxt = wpool.tile([128, max_icols], i16, tag="idxt")
                nc.sync.dma_start(idxt[:, 0:icl + ich],
                                  idx16[:, io:io + icl + ich])
                attw = wpool.tile([ED, max_cap], f32, tag="attw")
                nc.sync.dma_start(attw[:, 0:cw], attr_t[:, so:so + cw])
                wrb = wpool.tile([128, max_cap], f32, tag="wrb")
                nc.sync.dma_start(wrb[:, 0:cw],
                                  wrow[0:1, so:so + cw].to_broadcast([128, cw]))
                wcl = wpool.tile([128, max_nblk], f32, tag="wcl")
                nc.sync.dma_start(wcl[:, 0:nb], wcol[:, bo:bo + nb])
                evw = wpool.tile([128, max_nblk, 3], f32, tag="evw")
                nc.sync.dma_start(evw[:, 0:nb, :], evec_h[:, bo:bo + nb, :])

                # gathers (src features), chunked to <=GMAX indices per call
                GMAX = 1024
                def emit_gathers(dst, table_a, table_b, need_snorm):
                    for (tbl, n_idx, blk0, col0) in (
                            (table_a, cl, 0, 0), (table_b, chh, clb, icl)):
                        for g0 in range(0, n_idx, GMAX):
                            gn = min(GMAX, n_idx - g0)
                            gi = nc.gpsimd.dma_gather(
                                out_ap=dst[:, blk0 + g0 // 128:
                                           blk0 + (g0 + gn) // 128, :],
                                in_ap=tbl,
                                idxs_ap=idxt[:, col0 + g0 // 16:
                                             col0 + (g0 + gn) // 16],
                                num_idxs=gn, num_idxs_reg=gn,
                                elem_size=128)
                            if need_snorm:
                                # Tile doesn't track DRAM deps: order the
                                # gather after the last s_norm LN store.
                                tile.add_dep_helper(
                                    gi.ins, ln_stores[-1].ins,
                                    reason="gather after s_norm written")

                sg = wpool.tile([128, max_nblk, 128], f32, tag="sg")
                emit_gathers(sg, snA, snB, True)
                vg = wpool.tile([128, max_nblk, 128], f32, tag="vg")
                emit_gathers(vg, vpA, vpB, False)

                # unit edge vectors
                sq3 = wpool.tile([128, max_nblk, 3], f32, tag="sq3")
                nc.vector.tensor_tensor(sq3[:, 0:nb, :], evw[:, 0:nb, :],
                                        evw[:, 0:nb, :], op=OP.mult)
                n2 = wpool.tile([128, max_nblk, 1], f32, tag="n2")
                nc.vector.tensor_reduce(n2[:, 0:nb, :], sq3[:, 0:nb, :],
                                        axis=AX.X, op=OP.add)
                nrm = wpool.tile([128, max_nblk], f32, tag="nrm")
                nc.scalar.activation(nrm[:, 0:nb], n2[:, 0:nb, 0], AF.Sqrt,
                                     bias=0.0)
                nc.vector.tensor_scalar_max(nrm[:, 0:nb], nrm[:, 0:nb], 1e-8)
                invn = wpool.tile([128, max_nblk], f32, tag="invn")
                nc.vector.reciprocal(invn[:, 0:nb], nrm[:, 0:nb])
                unit = wpool.tile([128, max_nblk, 3], f32, tag="unit")
                nc.vector.tensor_tensor(
                    unit[:, 0:nb, :], evw[:, 0:nb, :],
                    invn[:, 0:nb, None].to_broadcast([128, nb, 3]), op=OP.mult)

                # window accumulator [128 targets, 225]
                acc = pacc.tile([128, 225], f32, tag="acc")
                first_mm = [True]

                ntile = (nb + TB - 1) // TB
                for t in range(ntile):
                    b0 = t * TB
                    tb = min(TB, nb - b0)
                    es = tb * 128
                    ts0 = b0 * 128

                    # transposed one-hot [targets, edges]
                    ohT = kpool.tile([128, TB * 128], f32, tag="ohT")
                    nc.vector.tensor_scalar(
                        ohT[:, 0:es], wrb[:, ts0:ts0 + es], iota_col[:],
                        None, op0=OP.is_equal)

                    # src feature transpose  [feat, edges]
                    srcT = kpool.tile([128, TB * 128], f32, tag="srcT")
                    for b in range(tb):
                        tp = pbig.tile([128, 128], f32, tag="big")
                        nc.tensor.transpose(tp[:], sg[:, b0 + b, :], ident[:])
                        if b % 2 == 0:
                            nc.scalar.copy(srcT[:, b * 128:(b + 1) * 128], tp[:])
                        else:
                            nc.vector.tensor_copy(srcT[:, b * 128:(b + 1) * 128],
                                                  tp[:])

                    # edge MLP layer 1 (accumulate tgt/src/attr blocks)
                    phs = []
                    for j, (mo, mw) in enumerate(CHK):
                        ph = pbig.tile([128, TB * 128], f32, tag="big")
                        nc.tensor.matmul(ph[0:mw, 0:es], gw[:, mo:mo + mw],
                                         ohT[:, 0:es], start=True, stop=False)
                        nc.tensor.matmul(ph[0:mw, 0:es], w1b[:, mo:mo + mw],
                                         srcT[:, 0:es], start=False, stop=False)
                        nc.tensor.matmul(ph[0:mw, 0:es], w1c[:, mo:mo + mw],
                                         attw[:, ts0:ts0 + es], start=False,
                                         stop=True)
                        phs.append(ph)

                    hs = []
                    for j, (mo, mw) in enumerate(CHK):
                        hsb = kpool.tile([128, TB * 128], f32, tag=f"hs{j}")
                        nc.scalar.activation(hsb[0:mw, 0:es], phs[j][0:mw, 0:es],
                                             AF.Silu, bias=b1c[0:mw, j:j + 1])
                        hs.append(hsb)

                    # process subtiles in pairs (PSUM budget)
                    for p2 in range((tb + 1) // 2):
                        sb0 = p2 * 2
                        ns = min(2, tb - sb0)
                        pu = pupd.tile([128, 2, 512], f32, tag="upd")
                        for b in range(ns):
                            col = (sb0 + b) * 128
                            for j, (mo, mw) in enumerate(CHK):
                                nc.tensor.matmul(
                                    pu[:, b, 0:EO],
                                    hs[j][0:mw, col:col + 128],
                                    w2p[0:mw, j * EO:(j + 1) * EO],
                                    start=(j == 0), stop=False)
                            # + b2 (rank-1 ones x b2)
                            nc.tensor.matmul(pu[:, b, 0:EO], ones_row[:],
                                             b2r[:], start=False, stop=True)

                        sgs = kpool.tile([128, 2, 128], f32, tag="sgs")
                        nc.scalar.activation(sgs[:, 0:ns, :],
                                             pu[:, 0:ns, 128:256], AF.Sigmoid)
                        vgs = kpool.tile([128, 2, 32], f32, tag="vgs")
                        nc.scalar.activation(vgs[:, 0:ns, :],
                                             pu[:, 0:ns, 256:288], AF.Sigmoid)
                        msg = kpool.tile([128, 2, 224], f32, tag="msg")
                        nc.vector.tensor_tensor(msg[:, 0:ns, 0:128],
                                                pu[:, 0:ns, 0:128],
                                                sgs[:, 0:ns, :], op=OP.mult)
                        nc.vector.tensor_tensor(
                            msg[:, 0:ns, 128:224],
                            vg[:, b0 + sb0:b0 + sb0 + ns, 0:F3],
                            vgs[:, 0:ns, None, :].to_broadcast([128, ns, 3, V]),
                            op=OP.mult)
                        dirm = kpool.tile([128, 2, 96], f32, tag="dirm")
                        nc.vector.tensor_tensor(
                            dirm[:, 0:ns, :].rearrange(
                                "p n (d v) -> p n d v", d=3),
                            unit[:, b0 + sb0:b0 + sb0 + ns, :, None]
                                .to_broadcast([128, ns, 3, V]),
                            pu[:, 0:ns, None, 288:320]
                                .to_broadcast([128, ns, 3, V]),
                            op=OP.mult)

                        for b in range(ns):
                            sb = sb0 + b
                            ohs = ohpool.tile([128, 128], f32, tag="ohs")
                            nc.vector.tensor_scalar(
                                ohs[:], iota_row[:],
                                wcl[:, b0 + sb:b0 + sb + 1], None,
                                op0=OP.is_equal)
                            st = first_mm[0]
                            first_mm[0] = False
                            # last matmul of the window closes the psum group
                            last = (t == ntile - 1 and sb == tb - 1)
                            nc.tensor.matmul(acc[:, 0:224], ohs[:],
                                             msg[:, b, :], start=st, stop=False)
                            nc.tensor.matmul(acc[:, 224:225], ohs[:],
                                             ones_col[:], start=False,
                                             stop=False)
                            nc.tensor.matmul(acc[:, 128:224], ohs[:],
                                             dirm[:, b, :], start=False,
                                             stop=last)

                # ---- node phase for this window ----------------------
                sc_t = kpool.tile([128, 128], f32, tag="sc_t")
                nc.sync.dma_start(sc_t[:], sc_local[w * 128:(w + 1) * 128, :])
                vc_t = kpool.tile([128, F3], f32, tag="vc_t")
                nc.sync.dma_start(vc_t[:], vec_local[w * 128:(w + 1) * 128, :])

                c1 = kpool.tile([128, 1], f32, tag="c1")
                nc.vector.tensor_scalar_max(c1[:], acc[:, 224:225], 1.0)
                invc = kpool.tile([128, 1], f32, tag="invc")
                nc.vector.reciprocal(invc[:], c1[:])
                s1 = kpool.tile([128, 128], f32, tag="s1")
                nc.vector.scalar_tensor_tensor(s1[:], acc[:, 0:128], invc[:],
                                               sc_t[:], op0=OP.mult, op1=OP.add)
                v1 = kpool.tile([128, F3], f32, tag="v1")
                nc.vector.scalar_tensor_tensor(v1[:], acc[:, 128:224], invc[:],
                                               vc_t[:], op0=OP.mult, op1=OP.add)

                # vnorm
                vsq = kpool.tile([128, F3], f32, tag="vsq")
                nc.vector.tensor_tensor(vsq[:], v1[:], v1[:], op=OP.mult)
                vn2 = kpool.tile([128, V], f32, tag="vn2")
                nc.vector.tensor_tensor(vn2[:], vsq[:, 0:V], vsq[:, V:2 * V],
                                        op=OP.add)
                nc.vector.tensor_tensor(vn2[:], vn2[:], vsq[:, 2 * V:3 * V],
                                        op=OP.add)
                nc.vector.tensor_scalar_max(vn2[:], vn2[:], 1e-8)
                vnm = kpool.tile([128, V], f32, tag="vnm")
                nc.scalar.activation(vnm[:], vn2[:], AF.Sqrt, bias=0.0)

                # LN(s1)
                lns = kpool.tile([128, 128], f32, tag="lns")
                ln_block(s1[:, None, :], 1, [lns[:]])

                # transpose node_in -> feature-major
                tpa = pbig.tile([128, 128], f32, tag="big")
                nc.tensor.transpose(tpa[:], lns[:], ident[:])
                nT = kpool.tile([128, 128], f32, tag="nT")
                nc.scalar.copy(nT[:], tpa[:])
                tpb = pbig.tile([128, 128], f32, tag="big")
                nc.tensor.transpose(tpb[0:V, :], vnm[:], ident[:])
                vT = kpool.tile([V, 128], f32, tag="vT")
                nc.scalar.copy(vT[:], tpb[0:V, :])

                # node MLP layer 1 (h2T feature-major)
                h2 = kpool.tile([128, 3, 128], f32, tag="h2")
                for j, (mo, mw) in enumerate(NCHK):
                    ph2 = pbig.tile([128, 128], f32, tag="big")
                    nc.tensor.matmul(ph2[0:mw, :],
                                     nw1p[:, mo:mo + mw], nT[:],
                                     start=True, stop=False)
                    nc.tensor.matmul(ph2[0:mw, :],
                                     nw1p[0:V, NH + mo:NH + mo + mw], vT[:],
                                     start=False, stop=True)
                    nc.scalar.activation(h2[0:mw, j, :], ph2[0:mw, :],
                                         AF.Silu, bias=nb1c[0:mw, j:j + 1])

                pnu = pbig.tile([128, NO], f32, tag="big")
                for j, (mo, mw) in enumerate(NCHK):
                    nc.tensor.matmul(pnu[:], h2[0:mw, j, :],
                                     nw2p[0:mw, j * NO:(j + 1) * NO],
                                     start=(j == 0), stop=False)
                nc.tensor.matmul(pnu[:], ones_row[:], nb2r[:], start=False,
                                 stop=True)

                s2 = kpool.tile([128, 128], f32, tag="s2")
                nc.vector.tensor_tensor(s2[:], s1[:], pnu[:, 0:128], op=OP.add)
                nc.sync.dma_start(s_out[w * 128:(w + 1) * 128, :], s2[:])

                th = kpool.tile([128, V], f32, tag="th")
                nc.scalar.activation(th[:], pnu[:, 128:160], AF.Tanh)
                vth = kpool.tile([128, F3], f32, tag="vth")
                nc.vector.tensor_tensor(
                    vth[:].rearrange("p (d v) -> p d v", d=3),
                    v1[:].rearrange("p (d v) -> p d v", d=3),
                    th[:, None, :].to_broadcast([128, 3, V]), op=OP.mult)
                v2 = kpool.tile([128, F3], f32, tag="v2")
                nc.vector.scalar_tensor_tensor(v2[:], vth[:], 0.1, v1[:],
                                               op0=OP.mult, op1=OP.add)
                nc.sync.dma_start(v_out[w * 128:(w + 1) * 128, :], v2[:])

    nc.finalize()
    return nc


# ---------------------------------------------------------------------------
# entry point
# ---------------------------------------------------------------------------

LAST_RESULT = None


def kernel(**inputs):
    global LAST_RESULT
    cfg = Cfg()
    in_maps, meta = host_prep(inputs, cfg)
    nc = build_program(cfg, meta)

    from concourse.bass_utils import run_bass_kernel_spmd
    res = run_bass_kernel_spmd(nc, in_maps, core_ids=list(range(cfg.ncores)))
    LAST_RESULT = res

    s_parts, v_parts = [], []
    for c in range(cfg.ncores):
        s_parts.append(res.results[c]["s_out"][0:cfg.npc])
        v_parts.append(res.results[c]["v_out"][0:cfg.npc])
    scalar = np.concatenate(s_parts, axis=0)
    vector = np.concatenate(v_parts, axis=0).reshape(cfg.N, 3, cfg.V)
    return scalar, vector
